# revision 1
# baseline (speedup 1.0000x reference)
"""Bass/Trainium2 kernel for nn_CrossSparseAttention.

Computes, for every (caption c, image i, word w):
    sims[c,i,w,r] = <caps[c,w], imgs[i,r]> / T   (masked by img/cap lengths)
    keep top-5 per row over r, masked softmax p, att = p @ imgs_m,
    out[i,c,w] = <att, caps[c,w]> / (||att|| + EPS), -1 where w >= cap_len.

Strategy (8 NeuronCores, caption axis sharded, imgs replicated):
  Phase A (rows = (c,w) on partitions, ragged (i, r<img_len) on free dim):
    fp32r matmuls give raw dots S; per-img max8 gives the top-8 of each
    (c,w,i) row in one DVE instruction -> rowmax m and threshold
    t' = (5th+6th)/2, both compact (P, 64).
  Phase B (flip layout: rows = (i,r) groups <= 128 partitions, (c,w) free):
    recompute S via fp32r matmul; subtract t' and m by folding small
    selector matmuls into the same PSUM accumulation; top-5 mask via
    saturated sigmoid on ACT; e = exp(10*(S-m)) * mask;
    B = e^T K e via host-precomputed per-img Gram block-diagonal matmul;
    per-img sums (s, A'=sum e*(S-m), B) via ones-matmuls accumulated into
    image-indexed stats PSUM across all groups.
  Final:  out = (A' + m*s) / (sqrt(B) + s*EPS)  on compact (64, NCW) tiles.
  Host scatters valid (c,w) columns into the full output and fills -1.

All algebra is exact w.r.t. the reference: with p = e/s,
  num = sum p*S = (A' + m*s)/s,  sq = B/s^2,  norm = sqrt(B)/s,
  out = num/(norm+EPS) = (A' + m*s)/(sqrt(B) + s*EPS).
"""

import numpy as np
import ml_dtypes
from contextlib import ExitStack

import concourse.bass as bass
import concourse.bacc as bacc
import concourse.tile as tile
import concourse.mybir as mybir
from concourse.bass_utils import run_bass_kernel_spmd

FP32 = mybir.dt.float32
FP32R = mybir.dt.float32r
BF16 = mybir.dt.bfloat16
ALU = mybir.AluOpType
ACTF = mybir.ActivationFunctionType

N_CORES = 8
N_IMG, R_PAD, D = 64, 36, 512
N_CAP, W_PAD = 64, 50
KNN = 5
INV_T = 10.0          # 1 / TEMPERATURE
EPS = -1e-8
MASK_VAL = -1.0
BIGSCALE = 1e12       # sigmoid(BIGSCALE * x) saturates to exact 0/1
KCHUNKS = 4           # 512 = 4 x 128 contraction chunks


def _pack(sizes, cap):
    """Greedy-pack consecutive items with sum(size) <= cap.
    Returns list of (start_item, end_item) (end exclusive)."""
    out = []
    s = 0
    while s < len(sizes):
        e = s
        tot = 0
        while e < len(sizes) and tot + sizes[e] <= cap:
            tot += sizes[e]
            e += 1
        out.append((s, e))
        s = e
    return out


def _build_program(lens, offs, NR, NCW, pchunks, groups, n_mt, mt_bounds,
                   debug_dump=False):
    """Build the SPMD bass program. All shape metadata is host-known."""
    nc = bacc.Bacc("TRN2", target_bir_lowering=False, debug=False)

    d_imgsT = nc.dram_tensor("imgsT", [D, NR], FP32R, kind="ExternalInput").ap()
    d_capsT = nc.dram_tensor("capsT", [D, NCW], FP32R, kind="ExternalInput").ap()
    d_imgsL = nc.dram_tensor("imgsL", [D, NR], FP32R, kind="ExternalInput").ap()
    d_capsL = nc.dram_tensor("capsL", [D, NCW], FP32R, kind="ExternalInput").ap()
    kbd_cols = sum(offs[e] - offs[s] for (s, e) in groups)
    d_kbd = nc.dram_tensor("kbd", [128, kbd_cols], FP32R, kind="ExternalInput").ap()
    esel_cols = kbd_cols
    d_esel = nc.dram_tensor("esel", [N_IMG, esel_cols], FP32R, kind="ExternalInput").ap()
    d_eselb = nc.dram_tensor("eselb", [N_IMG, esel_cols], BF16, kind="ExternalInput").ap()
    d_eselnb = nc.dram_tensor("eselnb", [N_IMG, esel_cols], BF16, kind="ExternalInput").ap()
    ones_cols = sum(ge for (_, ge) in groups)
    d_ones = nc.dram_tensor("onesbd", [128, ones_cols], FP32R, kind="ExternalInput").ap()
    d_ident = nc.dram_tensor("ident", [128, 128], FP32, kind="ExternalInput").ap()
    d_pbias = nc.dram_tensor("padbias", [128, max(1, len(groups))], FP32,
                             kind="ExternalInput").ap()
    d_out = nc.dram_tensor("out", [N_IMG, NCW], FP32, kind="ExternalOutput").ap()
    if debug_dump:
        d_dbg_S = nc.dram_tensor("dbg_S", [128, NR], FP32, kind="ExternalOutput").ap()
        d_dbg_mx = nc.dram_tensor("dbg_mx", [128, N_IMG * 8], FP32, kind="ExternalOutput").ap()
        d_dbg_r1 = nc.dram_tensor("dbg_r1", [N_IMG, NCW], FP32, kind="ExternalOutput").ap()
        d_dbg_r2 = nc.dram_tensor("dbg_r2", [N_IMG, NCW], FP32, kind="ExternalOutput").ap()
        d_dbg_mT = nc.dram_tensor("dbg_mT", [N_IMG, NCW], FP32, kind="ExternalOutput").ap()
        d_dbg_e = nc.dram_tensor("dbg_e", [128, NCW], FP32, kind="ExternalOutput").ap()
        d_dbg_ss = nc.dram_tensor("dbg_ss", [N_IMG, NCW], FP32, kind="ExternalOutput").ap()
        d_dbg_sa = nc.dram_tensor("dbg_sa", [N_IMG, NCW], FP32, kind="ExternalOutput").ap()
        d_dbg_sb = nc.dram_tensor("dbg_sb", [N_IMG, NCW], FP32, kind="ExternalOutput").ap()

    with tile.TileContext(nc) as tc, ExitStack() as ctx:
        const = ctx.enter_context(tc.tile_pool(name="const", bufs=1))
        # resident inputs
        imgsT = [const.tile([128, NR], FP32R, tag=f"imgsT{k}", name=f"imgsT{k}")
                 for k in range(KCHUNKS)]
        capsT = [const.tile([128, NCW], FP32R, tag=f"capsT{k}", name=f"capsT{k}")
                 for k in range(KCHUNKS)]
        imgsL = [const.tile([128, NR], FP32R, tag=f"imgsL{k}", name=f"imgsL{k}")
                 for k in range(KCHUNKS)]
        capsL = [const.tile([128, NCW], FP32R, tag=f"capsL{k}", name=f"capsL{k}")
                 for k in range(KCHUNKS)]
        for k in range(KCHUNKS):
            nc.sync.dma_start(imgsT[k][:], d_imgsT[128 * k:128 * (k + 1), :])
            nc.sync.dma_start(capsT[k][:], d_capsT[128 * k:128 * (k + 1), :])
            nc.sync.dma_start(imgsL[k][:], d_imgsL[128 * k:128 * (k + 1), :])
            nc.sync.dma_start(capsL[k][:], d_capsL[128 * k:128 * (k + 1), :])
        kbd = const.tile([128, kbd_cols], FP32R, tag="kbd")
        nc.sync.dma_start(kbd[:], d_kbd[:])
        esel = const.tile([N_IMG, esel_cols], FP32R, tag="esel")
        nc.sync.dma_start(esel[:], d_esel[:])
        eselb = const.tile([N_IMG, esel_cols], BF16, tag="eselb")
        nc.sync.dma_start(eselb[:], d_eselb[:])
        eselnb = const.tile([N_IMG, esel_cols], BF16, tag="eselnb")
        nc.sync.dma_start(eselnb[:], d_eselnb[:])
        onesbd = const.tile([128, ones_cols], FP32R, tag="ones")
        nc.sync.dma_start(onesbd[:], d_ones[:])
        ident = const.tile([128, 128], FP32, tag="ident")
        nc.sync.dma_start(ident[:], d_ident[:])
        pbias = const.tile([128, max(1, len(groups))], FP32, tag="pbias")
        nc.sync.dma_start(pbias[:], d_pbias[:])

        stat = ctx.enter_context(tc.tile_pool(name="stat", bufs=1))
        m_T = stat.tile([N_IMG, NCW], FP32, tag="m_T")          # = m_bf, fp32 view
        mT_bf = stat.tile([N_IMG, NCW], BF16, tag="mT_bf")      # bf16(m)
        nt_bf = stat.tile([N_IMG, NCW], BF16, tag="nt_bf")      # bf16(-t')

        phaseA = ExitStack()
        pool_sA = phaseA.enter_context(
            tc.tile_pool(name="psumA", bufs=len(pchunks) + 1, space="PSUM"))
        pool_tp = phaseA.enter_context(tc.tile_pool(name="psumT", bufs=2, space="PSUM"))
        pool_sb = phaseA.enter_context(tc.tile_pool(name="sbA", bufs=2))

        # ---------------- Phase A ----------------
        for mt in range(n_mt):
            lo, hi = mt_bounds[mt]
            mw = hi - lo
            psums = []
            for (ps, pe_) in pchunks:
                cs, ce = offs[ps], offs[pe_]
                p = pool_sA.tile([128, 512], FP32, tag="pA")
                psums.append((p, cs, ce))
                for k in range(KCHUNKS):
                    nc.tensor.matmul(
                        p[:mw, : ce - cs],
                        capsT[k][:, lo:hi],
                        imgsT[k][:, cs:ce],
                        start=(k == 0), stop=(k == KCHUNKS - 1),
                    )
            S = pool_sb.tile([128, NR], FP32, tag="S")
            for j, (p, cs, ce) in enumerate(psums):
                eng = nc.vector if j % 2 == 0 else nc.scalar
                if eng is nc.vector:
                    nc.vector.tensor_copy(S[:mw, cs:ce], p[:mw, : ce - cs])
                else:
                    nc.scalar.activation(S[:mw, cs:ce], p[:mw, : ce - cs], ACTF.Copy)
            mx = pool_sb.tile([128, N_IMG * 8], FP32, tag="mx")
            for i in range(N_IMG):
                nc.vector.max(mx[:mw, 8 * i:8 * i + 8],
                              S[:mw, offs[i]:offs[i] + lens[i]])
            if debug_dump and mt == 0:
                nc.sync.dma_start(d_dbg_S[:mw, :], S[:mw, :])
                nc.sync.dma_start(d_dbg_mx[:mw, :], mx[:mw, :])
            mx3 = mx[:mw, :].rearrange("p (i k) -> p i k", k=8)
            cmp = pool_sb.tile([128, 3 * N_IMG], FP32, tag="cmp")
            c3 = cmp[:mw, :].rearrange("p (j i) -> p j i", j=3)
            m_v = mx3[:, :, 4:5].rearrange("p i k -> p (i k)")
            m5 = mx3[:, :, 5:6].rearrange("p i k -> p (i k)")
            m0 = mx3[:, :, 0:1].rearrange("p i k -> p (i k)")
            t2 = c3[:, 0, :]
            # t2 = m4 + m5  (sum of 5th and 6th largest)
            nc.vector.tensor_add(t2, m_v, m5)
            # rhs1 = -0.5*t2 = -t'
            nc.vector.tensor_scalar(c3[:, 1, :], t2, -0.5, None, op0=ALU.mult)
            # transpose (mw, 64) compacts -> (64, mw); evac casts to bf16 so
            # the folded value and the added-back value are bit-identical
            # (bf16 passes through the PE's fp32r rounding unchanged).
            for src, dst in ((m0, mT_bf), (c3[:, 1, :], nt_bf)):
                pt = pool_tp.tile([N_IMG, 128], FP32, tag="pT")
                nc.tensor.transpose(pt[:, :mw], src, ident[:mw, :mw])
                nc.vector.tensor_copy(dst[:, lo:hi], pt[:, :mw])
            nc.vector.tensor_copy(m_T[:, lo:hi], mT_bf[:, lo:hi])

        if debug_dump:
            nc.sync.dma_start(d_dbg_mT[:], m_T[:])
        phaseA.close()

        # ---------------- Phase B ----------------
        pool_f = ctx.enter_context(tc.tile_pool(name="psumF", bufs=2, space="PSUM"))
        pool_ke = ctx.enter_context(tc.tile_pool(name="psumK", bufs=2, space="PSUM"))
        pool_st = ctx.enter_context(tc.tile_pool(name="psumS", bufs=1, space="PSUM"))
        pool_e = ctx.enter_context(tc.tile_pool(name="sbB", bufs=3))

        st_s = pool_st.tile([N_IMG, NCW], FP32, tag="st_s")
        st_a = pool_st.tile([N_IMG, NCW], FP32, tag="st_a")
        st_b = pool_st.tile([N_IMG, NCW], FP32, tag="st_b")
        # zero-fill all 64 stats partitions (and set has_written) before the
        # group accumulation: lhsT = an all-zero (2, 64) esel slice. Images
        # 32/33 are never in group 0, so esel[32:34, 0:64] is zero; partition
        # base 32 is a legal tile_position.
        for st in (st_s, st_a, st_b):
            nc.tensor.matmul(
                st[:N_IMG, :],
                esel[32:34, 0:N_IMG],
                capsT[0][32:34, :],
                start=True, stop=True, skip_group_check=True,
            )

        col = 0
        ocol = 0
        nG = len(groups)
        for g, (gs, ge) in enumerate(groups):
            rs, re = offs[gs], offs[ge]
            gr = re - rs
            Mg = ge  # stats land at partitions [0:ge) (image index base)
            p1 = pool_f.tile([128, NCW], FP32, tag="p1")
            # S in ~fp32 precision via 3-pass 11-bit hi/lo split
            for ki, (wa, wb) in enumerate(
                [(imgsT, capsT), (imgsT, capsL), (imgsL, capsT)]
            ):
                for k in range(KCHUNKS):
                    nc.tensor.matmul(
                        p1[:gr, :],
                        wa[k][:, rs:re],
                        wb[k][:],
                        start=(ki == 0 and k == 0), stop=False,
                    )
            # += -t'   (S - t' complete; close the accumulation group so the
            # mask can read PSUM -- stop is sim-only metadata)
            nc.tensor.matmul(
                p1[:gr, :],
                eselb[:, col:col + gr],
                nt_bf[:],
                start=False, stop=True,
            )
            mask = pool_e.tile([128, NCW], FP32, tag="mask")
            nc.scalar.activation(mask[:gr, :], p1[:gr, :], ACTF.Sigmoid,
                                 scale=BIGSCALE)
            # += +t' then -m  (now p1 = S - m_bf, with the t' fold cancelled
            # exactly: same tensor, negated selector)
            nc.tensor.matmul(
                p1[:gr, :],
                eselnb[:, col:col + gr],
                nt_bf[:],
                start=False, stop=True,
                skip_group_check=True,
            )
            nc.tensor.matmul(
                p1[:gr, :],
                eselnb[:, col:col + gr],
                mT_bf[:],
                start=False, stop=True,
                skip_group_check=True,
            )
            e0 = pool_e.tile([128, NCW], FP32, tag="e0")
            nc.scalar.activation(e0[:gr, :], p1[:gr, :], ACTF.Exp, scale=INV_T,
                                 bias=pbias[:gr, g:g + 1])
            e = pool_e.tile([128, NCW], FP32R, tag="e")
            nc.gpsimd.tensor_mul(e[:gr, :], e0[:gr, :], mask[:gr, :])
            eW = pool_e.tile([128, NCW], FP32R, tag="eW")
            nc.vector.scalar_tensor_tensor(
                eW[:gr, :], p1[:gr, :], 1.0, e[:gr, :],
                op0=ALU.bypass, op1=ALU.mult,
            )
            pk = pool_ke.tile([128, NCW], FP32, tag="pk")
            nc.tensor.matmul(
                pk[:gr, :],
                kbd[:gr, col:col + gr],
                e[:gr, :],
                start=True, stop=True,
            )
            if debug_dump and g == 0:
                nc.sync.dma_start(d_dbg_e[:gr, :], e[:gr, :].bitcast(FP32))
            eK = pool_e.tile([128, NCW], FP32R, tag="eK")
            nc.vector.scalar_tensor_tensor(
                eK[:gr, :], pk[:gr, :], 1.0, e[:gr, :],
                op0=ALU.bypass, op1=ALU.mult,
            )
            for st, rhs in ((st_s, e), (st_a, eW), (st_b, eK)):
                nc.tensor.matmul(
                    st[:Mg, :],
                    onesbd[:gr, ocol:ocol + Mg],
                    rhs[:gr, :],
                    start=False, stop=(g == nG - 1),
                    skip_group_check=True,
                )
            col += gr
            ocol += Mg

        # ---------------- Final ----------------
        fin = ctx.enter_context(tc.tile_pool(name="fin", bufs=1))
        if debug_dump:
            dss = fin.tile([N_IMG, NCW], FP32, tag="dss")
            nc.vector.tensor_copy(dss[:], st_s[:])
            nc.sync.dma_start(d_dbg_ss[:], dss[:])
            dsa = fin.tile([N_IMG, NCW], FP32, tag="dsa")
            nc.vector.tensor_copy(dsa[:], st_a[:])
            nc.sync.dma_start(d_dbg_sa[:], dsa[:])
            dsb = fin.tile([N_IMG, NCW], FP32, tag="dsb")
            nc.vector.tensor_copy(dsb[:], st_b[:])
            nc.sync.dma_start(d_dbg_sb[:], dsb[:])
        bcl = fin.tile([N_IMG, NCW], FP32, tag="bcl")
        nc.vector.tensor_scalar(bcl[:], st_b[:], 0.0, None, op0=ALU.max)
        sqB = fin.tile([N_IMG, NCW], FP32, tag="sqB")
        nc.scalar.activation(sqB[:], bcl[:], ACTF.Sqrt)
        n1a = fin.tile([N_IMG, NCW], FP32, tag="n1a")
        nc.vector.scalar_tensor_tensor(
            n1a[:], st_s[:], 1.0, m_T[:], op0=ALU.bypass, op1=ALU.mult
        )
        n1 = fin.tile([N_IMG, NCW], FP32, tag="n1")
        nc.vector.tensor_add(n1[:], n1a[:], st_a[:])
        d1 = fin.tile([N_IMG, NCW], FP32, tag="d1")
        nc.vector.scalar_tensor_tensor(
            d1[:], st_s[:], EPS, sqB[:], op0=ALU.mult, op1=ALU.add
        )
        rec = fin.tile([N_IMG, NCW], FP32, tag="rec")
        nc.vector.reciprocal(rec[:], d1[:])
        ov = fin.tile([N_IMG, NCW], FP32, tag="ov")
        nc.vector.tensor_mul(ov[:], n1[:], rec[:])
        nc.sync.dma_start(d_out[:], ov[:])

    nc.compile()
    return nc


def _rne11(x):
    """Round fp32 to 11 explicit mantissa bits (fp32r's internal rounding,
    round-to-nearest-even); returns (hi, lo) with x == hi + lo exactly."""
    u = np.ascontiguousarray(x, dtype=np.float32).view(np.uint32)
    hi = ((u + 0x7FF + ((u >> 12) & 1)) & 0xFFFFF000).view(np.float32)
    lo = (x - hi).astype(np.float32)
    return hi, lo


def kernel(imgs, caps, img_lens, cap_lens, _debug_dump=False):
    imgs = np.asarray(imgs, dtype=np.float32)
    caps = np.asarray(caps, dtype=np.float32)
    il = np.asarray(img_lens).astype(np.int64)
    cl = np.asarray(cap_lens).astype(np.int64)
    n_img, R, d = imgs.shape
    n_cap, W, _ = caps.shape

    lens = il.tolist()
    # fp32r matmuls need even N / 8B-aligned dst: pad each image's region
    # count to even in the ragged layout (pad columns are zero).
    lens_p = [l + (l & 1) for l in lens]
    offs = np.concatenate([[0], np.cumsum(lens_p)]).astype(int).tolist()
    NR = offs[-1]

    # ragged image layout (d, NR)
    imgsT = np.zeros((d, NR), dtype=np.float32)
    for i in range(n_img):
        imgsT[:, offs[i]:offs[i] + lens[i]] = imgs[i, :lens[i], :].T

    # phase-A psum img-chunks (cols <= 512) and flip groups (rows <= 128)
    pchunks = _pack(lens_p, 512)
    groups = _pack(lens_p, 128)

    # per-core caption columns
    caps_per = n_cap // N_CORES
    core_cols = []
    for k in range(N_CORES):
        cols = [(c, w) for c in range(caps_per * k, caps_per * (k + 1))
                for w in range(int(cl[c]))]
        core_cols.append(cols)
    NCW = max(len(c) for c in core_cols)
    NCW = max(NCW, 256)  # keep fp32r matmuls at full rate (N >= 256)
    NCW += NCW & 1       # even N for fp32r

    n_mt = max(1, -(-NCW // 128))
    mtw = -(-NCW // n_mt)
    mt_bounds = []
    lo = 0
    while lo < NCW:
        mt_bounds.append((lo, min(lo + mtw, NCW)))
        lo += mtw
    n_mt = len(mt_bounds)

    # block-diagonal Gram / selector / ones tensors (shared by all cores)
    kbd_cols = sum(offs[e] - offs[s] for (s, e) in groups)
    ones_cols = sum(ge for (_, ge) in groups)
    kbd = np.zeros((128, kbd_cols), dtype=np.float32)
    esel = np.zeros((n_img, kbd_cols), dtype=np.float32)
    onesbd = np.zeros((128, ones_cols), dtype=np.float32)
    padbias = np.zeros((128, max(1, len(groups))), dtype=np.float32)
    col = 0
    ocol = 0
    for g, (gs, ge) in enumerate(groups):
        r0 = offs[gs]
        for i in range(gs, ge):
            a = offs[i] - r0
            b = a + lens[i]           # real rows only; pad row stays zero
            X = imgs[i, :lens[i], :]
            kbd[a:b, col + a:col + b] = (X @ X.T).astype(np.float32)
            esel[i, col + a:col + b] = 1.0
            onesbd[a:b, ocol + i] = 1.0
            if lens_p[i] != lens[i]:
                padbias[b, g] = -1e9  # kill the pad row's exp in this group
        col += offs[ge] - r0
        ocol += ge
    ident = np.eye(128, dtype=np.float32)

    nc = _build_program(lens, offs, NR, NCW, pchunks, groups, n_mt, mt_bounds,
                        debug_dump=_debug_dump)

    imgsT_hi, imgsT_lo = _rne11(imgsT)
    eselb = esel.astype(ml_dtypes.bfloat16)
    eselnb = (-esel).astype(ml_dtypes.bfloat16)
    in_maps = []
    for k in range(N_CORES):
        capsT = np.zeros((d, NCW), dtype=np.float32)
        for j, (c, w) in enumerate(core_cols[k]):
            capsT[:, j] = caps[c, w, :]
        capsT_hi, capsT_lo = _rne11(capsT)
        in_maps.append({
            "imgsT": imgsT_hi, "capsT": capsT_hi,
            "imgsL": imgsT_lo, "capsL": capsT_lo,
            "kbd": kbd, "esel": esel, "eselb": eselb, "eselnb": eselnb,
            "onesbd": onesbd, "ident": ident, "padbias": padbias,
        })

    if _debug_dump:
        res = run_bass_kernel_spmd(nc, in_maps[:1], core_ids=[0])
        kernel._dbg = res.results[0]
        kernel._meta = dict(lens=lens, lens_p=lens_p, offs=offs, NCW=NCW,
                            groups=groups, core_cols=core_cols)
        out = np.full((n_img, n_cap, W), MASK_VAL, dtype=np.float32)
        dev = res.results[0]["out"]
        cols = core_cols[0]
        cc = np.array([c for c, _ in cols]); ww = np.array([w for _, w in cols])
        out[:, cc, ww] = dev[:, :len(cols)]
        return out
    res = run_bass_kernel_spmd(nc, in_maps, core_ids=list(range(N_CORES)))

    out = np.full((n_img, n_cap, W), MASK_VAL, dtype=np.float32)
    for k in range(N_CORES):
        dev = res.results[k]["out"]
        cols = core_cols[k]
        if cols:
            cc = np.array([c for c, _ in cols])
            ww = np.array([w for _, w in cols])
            out[:, cc, ww] = dev[:, :len(cols)]
    return out

